# revision 1
# baseline (speedup 1.0000x reference)
"""Trainium2 Bass kernel for nn_ConvAttention (dwconv3x3->BN->GELU->1x1 conv
q/k/v branches, 8-head attention over 32x32 tokens, 1x1 out-proj, BN).

Sharding: data-parallel over batch B=8 across the 8 NeuronCores (one image
per core). The two training-mode BatchNorms couple cores across the batch:
  - the q/k/v-branch BN stats are computed on the HOST, exactly, from the
    inputs (the depthwise conv is recomputed cheaply in numpy just for the
    statistics; the device computes the conv for the actual data path), so
    the device kernel needs no cross-core communication at all;
  - the final BN is applied on the host after gathering (elementwise).

Device per-core pipeline (single NEFF launch):
  x -> pad -> bf16 -> 9 accumulated diagonal matmuls per 128-channel block
  (depthwise conv on the PE) -> fused scale/bias+GELU on ACT (folded BN) ->
  fp32 pointwise matmuls (M=96 head-pairs) -> per-head attention:
  S^T chunks = k_chunk^T q (bf16), P^T = exp(scale*S^T) on ACT -> bf16,
  O = [v^T | 1]^T P^T accumulated over chunks (row 48 = softmax denom),
  divide via reciprocal + ones-broadcast matmul -> fp32 out-projection
  accumulated over heads.
"""

import sys
import types

sys.path.insert(0, "/opt/trn_rl_repo")

import numpy as np
import ml_dtypes

import concourse.bass as bass
import concourse.mybir as mybir
import concourse.tile as tile
from concourse.bass_utils import run_bass_kernel_spmd
from concourse.masks import make_identity

BF16 = ml_dtypes.bfloat16
F32 = mybir.dt.float32
BF = mybir.dt.bfloat16

B, C, H, W = 8, 384, 32, 32
N = H * W
HEADS, HD = 8, 48
SCALE = float(HD ** -0.5)
NBLK = C // 128          # 3 channel blocks
NPAIR = HEADS // 2       # 4 head pairs (M=96 pointwise blocks)
EPS = 1e-5

_GELU = mybir.ActivationFunctionType.Gelu
GELU_FUNC = [_GELU]  # sim_check overrides (CoreSim lacks Gelu)
_EXP = mybir.ActivationFunctionType.Exp
_LN = mybir.ActivationFunctionType.Ln


# ---------------------------------------------------------------- wait split
def _split_excess_waits(nc, max_waits=1):
    """Old walrus rejects >1 sync wait per instruction; hoist extras onto
    NoOps inserted just before, on the same engine (queue order preserved)."""
    n = 0
    for f in nc.m.functions:
        for bb in f.blocks:
            out, changed = [], False
            for inst in bb.instructions:
                si = inst.sync_info
                waits = list(si.on_wait) if si is not None else []
                if len(waits) > max_waits:
                    excess, keep = waits[:-max_waits], waits[-max_waits:]
                    for j, w in enumerate(excess):
                        nop = mybir.InstNoOp(
                            name=f"WSPLIT-{inst.name}-{j}", ins=[], outs=[])
                        nop.engine = inst.engine
                        nop.sync_info = mybir.SyncInfo(on_wait=[w], on_update=[])
                        out.append(nop)
                        n += 1
                    inst.sync_info = mybir.SyncInfo(
                        on_wait=keep, on_update=list(si.on_update))
                    changed = True
                out.append(inst)
            if changed:
                bb.instructions = out
    return n


# ---------------------------------------------------------------- builder
def build_kernel(split_waits=True):
    nc = bass.Bass("TRN2", target_bir_lowering=False, debug=False)

    x_d = nc.dram_tensor("x", [C, H, W], F32, kind="ExternalInput").ap()
    diag_d = nc.dram_tensor("diags", [3, NBLK, 9, 128, 128], BF,
                            kind="ExternalInput").ap()
    A_d = nc.dram_tensor("scaleA", [3, C, 1], F32, kind="ExternalInput").ap()
    D_d = nc.dram_tensor("biasD", [3, C, 1], F32, kind="ExternalInput").ap()
    pwT_d = nc.dram_tensor("pwT", [3, C, NPAIR, 112], F32,
                       kind="ExternalInput").ap()
    woT_d = nc.dram_tensor("woT", [HEADS, HD, C], F32,
                           kind="ExternalInput").ap()
    out_d = nc.dram_tensor("out", [C, N], F32, kind="ExternalOutput").ap()

    with tile.TileContext(nc) as tc:
        from contextlib import ExitStack
        ctx = ExitStack()
        with ctx:
            cpool = ctx.enter_context(tc.tile_pool(name="consts", bufs=1))
            xpool = ctx.enter_context(tc.tile_pool(name="xin", bufs=2))
            padpool = ctx.enter_context(tc.tile_pool(name="pads", bufs=1))
            yhpool = ctx.enter_context(tc.tile_pool(name="yh", bufs=1))
            qkvpool = ctx.enter_context(tc.tile_pool(name="qkv", bufs=1))
            vtpool = ctx.enter_context(tc.tile_pool(name="vt", bufs=1))
            ptpool = ctx.enter_context(tc.tile_pool(name="pt", bufs=2))
            opool = ctx.enter_context(tc.tile_pool(name="osb", bufs=1))
            dpool = ctx.enter_context(tc.tile_pool(name="div", bufs=2))
            outpool = ctx.enter_context(tc.tile_pool(name="outsb", bufs=2))

            # PSUM is only 8 banks of (128, 2KB); everything must fit in
            # 4 slots of 2 banks: 2 rotating "flow" slots (S chunks,
            # transposes, broadcasts) + 2 rotating "acc" slots (conv/pw/
            # attention-O/out-proj accumulators).
            ps_acc = ctx.enter_context(
                tc.tile_pool(name="ps_acc", bufs=2, space="PSUM"))
            ps_flow = ctx.enter_context(
                tc.tile_pool(name="ps_flow", bufs=2, space="PSUM"))

            # ---------------- constants
            ident = cpool.tile([128, 128], BF, tag="ident")
            make_identity(nc, ident[:])
            negones = cpool.tile([65, 48], F32, tag="negones")
            nc.gpsimd.memset(negones[:], -1.0)

            # ---------------- depthwise conv + BN + GELU
            xpad = {}
            for blk in range(NBLK):
                xt = xpool.tile([128, H, W], F32)
                nc.sync.dma_start(xt[:], x_d[blk * 128:(blk + 1) * 128])
                xp = padpool.tile([128, H + 2, W + 2], BF, tag=f"xpad{blk}")
                nc.gpsimd.memset(xp[:], 0.0)
                nc.vector.tensor_copy(xp[:, 1:H + 1, 1:W + 1], xt[:])
                xpad[blk] = xp

            diag_t = {}
            for br in range(3):
                for blk in range(NBLK):
                    for tap in range(9):
                        t = cpool.tile([128, 128], BF,
                                       tag=f"diag{br}_{blk}_{tap}")
                        nc.sync.dma_start(t[:], diag_d[br, blk, tap])
                        diag_t[(br, blk, tap)] = t
            A_t, D_t = {}, {}
            for br in range(3):
                for blk in range(NBLK):
                    a = cpool.tile([128, 1], F32, tag=f"A{br}_{blk}")
                    d = cpool.tile([128, 1], F32, tag=f"D{br}_{blk}")
                    nc.sync.dma_start(
                        a[:], A_d[br, blk * 128:(blk + 1) * 128, :])
                    nc.sync.dma_start(
                        d[:], D_d[br, blk * 128:(blk + 1) * 128, :])
                    A_t[(br, blk)] = a
                    D_t[(br, blk)] = d
            pwT_t = {}
            for br in range(3):
                for kc in range(NBLK):
                    t = cpool.tile([128, NPAIR, 112], F32,
                                   tag=f"pwT{br}_{kc}")
                    nc.sync.dma_start(
                        t[:], pwT_d[br, kc * 128:(kc + 1) * 128, :])
                    pwT_t[(br, kc)] = t
            woT_t = {}
            for h in range(HEADS):
                t = cpool.tile([HD, C], F32, tag=f"woT{h}")
                nc.sync.dma_start(t[:], woT_d[h])
                woT_t[h] = t

            yh_t = {}
            for br in range(3):
                for blk in range(NBLK):
                    py = ps_acc.tile([128, N], F32, tag="acc")
                    for tap in range(9):
                        di, dj = tap // 3, tap % 3
                        for hf in range(2):
                            nc.tensor.matmul(
                                py[:, hf * 512:(hf + 1) * 512],
                                diag_t[(br, blk, tap)][:],
                                xpad[blk][:, di + 16 * hf:di + 16 * hf + 16,
                                          dj:dj + W],
                                start=(tap == 0), stop=(tap == 8))
                    yh = yhpool.tile([128, N], F32, tag=f"yh{br}_{blk}")
                    nc.scalar.activation(
                        yh[:], py[:], GELU_FUNC[0],
                        bias=D_t[(br, blk)][:], scale=A_t[(br, blk)][:])
                    yh_t[(br, blk)] = yh

            # ---------------- pointwise (fp32, M=96 head-pairs)
            qkv_sb = {}
            for pair in range(NPAIR):
                for br in range(3):
                    pp = ps_acc.tile([112, N], F32, tag="acc")
                    for kc in range(NBLK):
                        lhsT = pwT_t[(br, kc)][:, pair, :]
                        for nch in range(2):
                            nc.tensor.matmul(
                                pp[:, nch * 512:(nch + 1) * 512],
                                lhsT,
                                yh_t[(br, kc)][:, nch * 512:(nch + 1) * 512],
                                start=(kc == 0), stop=(kc == NBLK - 1))
                    sb = qkvpool.tile([112, N], BF, tag=f"qkv{br}_{pair}")
                    nc.vector.tensor_copy(sb[:], pp[:])
                    qkv_sb[(br, pair)] = sb

            # ---------------- v^T tiles (per head, per 128-chunk of m)
            vT_t = {}
            for pair in range(NPAIR):
                for hh in range(2):
                    h = 2 * pair + hh
                    off = 64 * hh
                    for j in range(8):
                        pt = ps_flow.tile([128, 48], BF, tag="flow")
                        nc.tensor.transpose(
                            pt[:],
                            qkv_sb[(2, pair)][off:off + 48,
                                              j * 128:(j + 1) * 128],
                            ident[off:off + 48, off:off + 48])
                        vt = vtpool.tile([128, 65], BF, tag=f"vt{h}_{j}")
                        nc.vector.tensor_copy(vt[:, 0:48], pt[:])
                        nc.gpsimd.memset(vt[:, 48:64], 0.0)
                        nc.gpsimd.memset(vt[:, 64:65], 1.0)
                        vT_t[(h, j)] = vt

            # ---------------- attention
            O_sb = {}
            for pair in range(NPAIR):
                for hh in range(2):
                    h = 2 * pair + hh
                    off = 64 * hh
                    q_ap = qkv_sb[(0, pair)][off:off + 48, :]
                    k_sb = qkv_sb[(1, pair)]
                    pO = ps_acc.tile([65, N], F32, tag="acc")
                    for j in range(8):
                        pS = ps_flow.tile([128, N], F32, tag="flow")
                        for nch in range(2):
                            nc.tensor.matmul(
                                pS[:, nch * 512:(nch + 1) * 512],
                                k_sb[off:off + 48, j * 128:(j + 1) * 128],
                                q_ap[:, nch * 512:(nch + 1) * 512],
                                start=True, stop=True)
                        pt = ptpool.tile([128, N], BF)
                        nc.scalar.activation(
                            pt[:], pS[:], _EXP, bias=0.0, scale=SCALE)
                        for nch in range(2):
                            nc.tensor.matmul(
                                pO[:, nch * 512:(nch + 1) * 512],
                                vT_t[(h, j)][:],
                                pt[:, nch * 512:(nch + 1) * 512],
                                start=(j == 0), stop=(j == 7))
                    # rows 0..47 /= row 64 via O * exp(-ln r):
                    # ACT Ln on the denom row, -1s-matmul broadcast to 48
                    # partitions, ACT Exp, one DVE multiply.
                    lnr = dpool.tile([65, N], F32, tag="rinv")
                    nc.scalar.activation(
                        lnr[64:65, :], pO[64:65, :], _LN, bias=0.0, scale=1.0)
                    pb = ps_flow.tile([48, N], F32, tag="flow")
                    for nch in range(2):
                        nc.tensor.matmul(
                            pb[:, nch * 512:(nch + 1) * 512],
                            negones[64:65, :],
                            lnr[64:65, nch * 512:(nch + 1) * 512],
                            start=True, stop=True)
                    bc = dpool.tile([48, N], F32, tag="bc")
                    nc.scalar.activation(bc[:], pb[:], _EXP, bias=0.0, scale=1.0)
                    osb = opool.tile([48, N], F32, tag=f"O{h}")
                    nc.vector.tensor_mul(osb[:], pO[0:48, :], bc[:])
                    O_sb[h] = osb

            # ---------------- out projection (fp32, K=48 per head)
            for m in range(NBLK):
                po = ps_acc.tile([128, N], F32, tag="acc")
                for h in range(HEADS):
                    lhsT = woT_t[h][:, m * 128:(m + 1) * 128]
                    for nch in range(2):
                        nc.tensor.matmul(
                            po[:, nch * 512:(nch + 1) * 512],
                            lhsT,
                            O_sb[h][:, nch * 512:(nch + 1) * 512],
                            start=(h == 0), stop=(h == HEADS - 1))
                ob = outpool.tile([128, N], F32)
                nc.vector.tensor_copy(ob[:], po[:])
                nc.sync.dma_start(out_d[m * 128:(m + 1) * 128, :], ob[:])

    if split_waits:
        _split_excess_waits(nc)
    return nc


_NC_CACHE = {}


def _get_nc():
    if "nc" not in _NC_CACHE:
        _NC_CACHE["nc"] = build_kernel()
    return _NC_CACHE["nc"]


# ---------------------------------------------------------------- host prep
def _conv_dw_np(x, dw):
    # x: (B, C, H, W) f32; dw: (C, 3, 3). padding=1 depthwise conv.
    Bx, Cx, Hx, Wx = x.shape
    xp = np.zeros((Bx, Cx, Hx + 2, Wx + 2), np.float32)
    xp[:, :, 1:Hx + 1, 1:Wx + 1] = x
    y = np.zeros((Bx, Cx, Hx, Wx), np.float32)
    for i in range(3):
        for j in range(3):
            y += dw[None, :, i, j, None, None] * \
                xp[:, :, i:i + Hx, j:j + Wx]
    return y


def _host_prep(inputs):
    x = np.ascontiguousarray(np.asarray(inputs["x"], np.float32))
    diags = np.zeros((3, NBLK, 9, 128, 128), BF16)
    A = np.zeros((3, C, 1), np.float32)
    D = np.zeros((3, C, 1), np.float32)
    pwT = np.zeros((3, C, NPAIR, 112), np.float32)
    idx = np.arange(128)
    for br, p in enumerate(["q", "k", "v"]):
        dw = np.asarray(inputs[f"dw_{p}"], np.float32).reshape(C, 3, 3)
        dwb = dw.astype(BF16).astype(np.float32)
        y = _conv_dw_np(x, dwb)          # matches device conv (bf16 weights)
        m = y.astype(np.float64).mean(axis=(0, 2, 3))
        v = y.astype(np.float64).var(axis=(0, 2, 3))
        g = np.asarray(inputs[f"g_{p}"], np.float64)
        bb = np.asarray(inputs[f"b_{p}"], np.float64)
        a = g / np.sqrt(v + EPS)
        A[br, :, 0] = a.astype(np.float32)
        D[br, :, 0] = (bb - m * a).astype(np.float32)
        for blk in range(NBLK):
            for tap in range(9):
                diags[br, blk, tap, idx, idx] = \
                    dwb[blk * 128:(blk + 1) * 128, tap // 3, tap % 3]
        pwt = np.asarray(inputs[f"pw_{p}"], np.float32).T  # (c_in, c_out)
        for pair in range(NPAIR):
            pwT[br, :, pair, 0:48] = pwt[:, (2 * pair) * 48:(2 * pair + 1) * 48]
            pwT[br, :, pair, 64:112] = pwt[:, (2 * pair + 1) * 48:(2 * pair + 2) * 48]
    w_out = np.asarray(inputs["w_out"], np.float32)
    woT = np.ascontiguousarray(
        w_out.T.reshape(HEADS, HD, C)).astype(np.float32)
    return x, diags, A, D, pwT, woT


def kernel(**inputs) -> np.ndarray:
    x, diags, A, D, pwT, woT = _host_prep(inputs)
    nc = _get_nc()
    in_maps = []
    for b in range(B):
        in_maps.append({
            "x": np.ascontiguousarray(x[b]),
            "diags": diags,
            "scaleA": A,
            "biasD": D,
            "pwT": pwT,
            "woT": woT,
        })
    res = run_bass_kernel_spmd(nc, in_maps, list(range(B)))
    out = np.stack([res.results[b]["out"] for b in range(B)])  # (B, C, N)

    o64 = out.astype(np.float64)
    m = o64.mean(axis=(0, 2))
    v = o64.var(axis=(0, 2))
    g = np.asarray(inputs["g_out"], np.float64)
    bb = np.asarray(inputs["b_out"], np.float64)
    res_f = (o64 - m[None, :, None]) / np.sqrt(v + EPS)[None, :, None] * \
        g[None, :, None] + bb[None, :, None]
    return res_f.reshape(B, C, H, W).astype(np.float32)



# revision 5
# speedup vs baseline: 1.4445x; 1.4445x over previous
"""Trainium2 Bass kernel for nn_ConvAttention (dwconv3x3->BN->GELU->1x1 conv
q/k/v branches, 8-head attention over 32x32 tokens, 1x1 out-proj, BN).

Sharding: data-parallel over batch B=8 across the 8 NeuronCores (one image
per core). The two training-mode BatchNorms couple cores across the batch:
  - the q/k/v-branch BN stats are computed on the HOST, exactly, from the
    inputs (the depthwise conv is recomputed cheaply in numpy just for the
    statistics; the device computes the conv for the actual data path), so
    the device kernel needs no cross-core communication at all;
  - the final BN is applied on the host after gathering (elementwise).

Device per-core pipeline (single NEFF launch), all matmuls bf16:
  x -> pad -> bf16 -> 9 accumulated diagonal matmuls per 128-channel block
  (depthwise conv on the PE) -> fused scale/bias+GELU on ACT (folded BN,
  bf16 out) -> bf16 pointwise matmuls. The v branch's pointwise runs in
  transposed form (lhsT = GELU output token chunk) so v^T tiles fall out of
  the matmul directly, with a zeros/ones tail appended per head for the
  softmax denominator row. Per head: S^T chunks = k_chunk^T q, P^T =
  exp(scale*S^T) on ACT -> bf16, O = [v^T|0|1]^T P^T accumulated over
  chunks (row 64 = denom), division via DVE reciprocal_approx_fast +
  bf16 ones-broadcast matmul + DVE multiply -> bf16 out-projection.
"""

import sys

sys.path.insert(0, "/opt/trn_rl_repo")

import numpy as np
import ml_dtypes

import concourse.bass as bass
import concourse.mybir as mybir
import concourse.tile as tile
from concourse.bass_utils import run_bass_kernel_spmd

BF16 = ml_dtypes.bfloat16
F32 = mybir.dt.float32
BF = mybir.dt.bfloat16
FP16 = mybir.dt.float16

B, C, H, W = 8, 384, 32, 32
N = H * W
HEADS, HD = 8, 48
SCALE = float(HD ** -0.5)
NBLK = C // 128          # 3 channel blocks
NPAIR = HEADS // 2       # 4 head pairs (M=112 pointwise blocks)
EPS = 1e-5
VSTR = 66                # per-head stride in the v^T tile (48 v + 16 z + 1 one + pad)

_GELU = mybir.ActivationFunctionType.Gelu
GELU_FUNC = [_GELU]  # sim_check overrides (CoreSim lacks Gelu)
_EXP = mybir.ActivationFunctionType.Exp


# ---------------------------------------------------------------- wait split
def _split_excess_waits(nc, max_waits=1):
    """Old walrus rejects >1 sync wait per instruction; hoist extras onto
    NoOps inserted just before, on the same engine (queue order preserved)."""
    n = 0
    for f in nc.m.functions:
        for bb in f.blocks:
            out, changed = [], False
            for inst in bb.instructions:
                si = inst.sync_info
                waits = list(si.on_wait) if si is not None else []
                if len(waits) > max_waits:
                    excess, keep = waits[:-max_waits], waits[-max_waits:]
                    for j, w in enumerate(excess):
                        nop = mybir.InstNoOp(
                            name=f"WSPLIT-{inst.name}-{j}", ins=[], outs=[])
                        nop.engine = inst.engine
                        nop.sync_info = mybir.SyncInfo(on_wait=[w], on_update=[])
                        out.append(nop)
                        n += 1
                    inst.sync_info = mybir.SyncInfo(
                        on_wait=keep, on_update=list(si.on_update))
                    changed = True
                out.append(inst)
            if changed:
                bb.instructions = out
    return n


# ---------------------------------------------------------------- builder
def build_kernel(split_waits=True):
    nc = bass.Bass("TRN2", target_bir_lowering=False, debug=False)

    x_d = nc.dram_tensor("x", [C, H, W], F32, kind="ExternalInput").ap()
    diag_d = nc.dram_tensor("diags", [3, NBLK, 128, 9, 128], BF,
                            kind="ExternalInput").ap()
    A_d = nc.dram_tensor("scaleA", [3, C, 1], F32, kind="ExternalInput").ap()
    D_d = nc.dram_tensor("biasD", [3, C, 1], F32, kind="ExternalInput").ap()
    pwT_d = nc.dram_tensor("pwT", [2, C, NPAIR, 112], BF,
                           kind="ExternalInput").ap()
    pwvT_d = nc.dram_tensor("pwvT", [NBLK, 128, C], BF,
                            kind="ExternalInput").ap()
    woT_d = nc.dram_tensor("woT", [HEADS, HD, C], FP16,
                           kind="ExternalInput").ap()
    out_d = nc.dram_tensor("out", [C, N], F32, kind="ExternalOutput").ap()

    with tile.TileContext(nc) as tc:
        from contextlib import ExitStack
        ctx = ExitStack()
        with ctx:
            cpool = ctx.enter_context(tc.tile_pool(name="consts", bufs=1))
            xpool = ctx.enter_context(tc.tile_pool(name="xin", bufs=2))
            padpool = ctx.enter_context(tc.tile_pool(name="pads", bufs=1))
            yhpool = ctx.enter_context(tc.tile_pool(name="yh", bufs=1))
            qkvpool = ctx.enter_context(tc.tile_pool(name="qkv", bufs=1))
            vtpool = ctx.enter_context(tc.tile_pool(name="vt", bufs=1))
            ptpool = ctx.enter_context(tc.tile_pool(name="pt", bufs=3))
            opool = ctx.enter_context(tc.tile_pool(name="osb", bufs=1))
            dpool = ctx.enter_context(tc.tile_pool(name="div", bufs=2))
            outpool = ctx.enter_context(tc.tile_pool(name="outsb", bufs=2))

            # PSUM: 8 banks of (128, 2KB); 2 rotating "flow" slots (S chunks,
            # div broadcasts) + 2 rotating "acc" slots (conv/pw/attention-O/
            # out-proj accumulators), 2 banks each.
            ps_acc = ctx.enter_context(
                tc.tile_pool(name="ps_acc", bufs=2, space="PSUM"))
            ps_flow = ctx.enter_context(
                tc.tile_pool(name="ps_flow", bufs=2, space="PSUM"))

            # ---------------- constants
            # ones row lives at partition 64 so it can pair with the softmax
            # denominator row (pO row 64) as matmul operands (32-aligned base).
            ones1 = cpool.tile([65, HD], FP16, tag="ones1")
            nc.gpsimd.memset(ones1[64:65, :], 1.0)

            # ---------------- inputs: x (pad to bf16), per-branch consts
            xpad = {}
            for blk in range(NBLK):
                xt = xpool.tile([128, H, W], F32)
                nc.sync.dma_start(xt[:], x_d[blk * 128:(blk + 1) * 128])
                xp = padpool.tile([128, H + 2, W + 2], BF, tag=f"xpad{blk}")
                nc.gpsimd.memset(xp[:], 0.0)
                nc.vector.tensor_copy(xp[:, 1:H + 1, 1:W + 1], xt[:])
                xpad[blk] = xp

            # branch emission order: v first so attention can start earliest
            BR_ORDER = (2, 0, 1)
            diag_t, A_t, D_t = {}, {}, {}
            for br in BR_ORDER:
                for blk in range(NBLK):
                    t = cpool.tile([128, 9, 128], BF, tag=f"diag{br}_{blk}")
                    nc.sync.dma_start(t[:], diag_d[br, blk])
                    diag_t[(br, blk)] = t
                    a = cpool.tile([128, 1], F32, tag=f"A{br}_{blk}")
                    d = cpool.tile([128, 1], F32, tag=f"D{br}_{blk}")
                    nc.sync.dma_start(
                        a[:], A_d[br, blk * 128:(blk + 1) * 128, :])
                    nc.sync.dma_start(
                        d[:], D_d[br, blk * 128:(blk + 1) * 128, :])
                    A_t[(br, blk)] = a
                    D_t[(br, blk)] = d
            pwvT_t = {}
            for kc in range(NBLK):
                t = cpool.tile([128, C], BF, tag=f"pwvT{kc}")
                nc.sync.dma_start(t[:], pwvT_d[kc])
                pwvT_t[kc] = t
            pwT_t = {}
            for br in range(2):
                for kc in range(NBLK):
                    t = cpool.tile([128, NPAIR, 112], BF,
                                   tag=f"pwT{br}_{kc}")
                    nc.sync.dma_start(
                        t[:], pwT_d[br, kc * 128:(kc + 1) * 128, :])
                    pwT_t[(br, kc)] = t
            woT_t = {}
            for h in range(HEADS):
                t = cpool.tile([HD, C], FP16, tag=f"woT{h}")
                nc.sync.dma_start(t[:], woT_d[h])
                woT_t[h] = t

            # ---------------- depthwise conv + BN + GELU (bf16 out)
            yh_t = {}

            def conv_branch(br):
                for blk in range(NBLK):
                    py = ps_acc.tile([128, N], F32, tag="acc")
                    dt = diag_t[(br, blk)]
                    for tap in range(9):
                        di, dj = tap // 3, tap % 3
                        for hf in range(2):
                            nc.tensor.matmul(
                                py[:, hf * 512:(hf + 1) * 512],
                                dt[:, tap, :],
                                xpad[blk][:, di + 16 * hf:di + 16 * hf + 16,
                                          dj:dj + W],
                                start=(tap == 0), stop=(tap == 8))
                    yh = yhpool.tile([128, N], BF, tag=f"yh{br}_{blk}")
                    nc.scalar.activation(
                        yh[:], py[:], GELU_FUNC[0],
                        bias=D_t[(br, blk)][:], scale=A_t[(br, blk)][:])
                    yh_t[(br, blk)] = yh

            # ---------------- v pointwise, transposed: vt[j] rows = tokens
            conv_branch(2)
            vt_all = {}
            for j in range(8):
                pv = ps_acc.tile([128, HEADS, HD], F32, tag="acc")
                for kc in range(NBLK):
                    nc.tensor.matmul(
                        pv[:, :, :],
                        yh_t[(2, kc)][:, j * 128:(j + 1) * 128],
                        pwvT_t[kc][:],
                        start=(kc == 0), stop=(kc == NBLK - 1))
                vt = vtpool.tile([128, HEADS, VSTR], BF, tag=f"vt{j}")
                nc.gpsimd.memset(vt[:, :, HD:64], 0.0)
                nc.gpsimd.memset(vt[:, :, 64:65], 1.0)
                nc.vector.tensor_copy(vt[:, :, 0:HD], pv[:])
                vt_all[j] = vt

            # ---------------- q,k pointwise (bf16, M=112 head-pairs)
            conv_branch(0)
            conv_branch(1)
            qkv_sb = {}

            def pw_pair(br, pair):
                pp = ps_acc.tile([112, N], F32, tag="acc")
                for kc in range(NBLK):
                    lhsT = pwT_t[(br, kc)][:, pair, :]
                    for nch in range(2):
                        nc.tensor.matmul(
                            pp[:, nch * 512:(nch + 1) * 512],
                            lhsT,
                            yh_t[(br, kc)][:, nch * 512:(nch + 1) * 512],
                            start=(kc == 0), stop=(kc == NBLK - 1))
                sb = qkvpool.tile([112, N], BF, tag=f"qkv{br}_{pair}")
                nc.vector.tensor_copy(sb[:], pp[:])
                qkv_sb[(br, pair)] = sb

            # ---------------- attention (interleave pw of later pairs)
            O_sb = {}
            for pair in range(NPAIR):
                pw_pair(0, pair)
                pw_pair(1, pair)
                for hh in range(2):
                    h = 2 * pair + hh
                    off = 64 * hh
                    q_ap = qkv_sb[(0, pair)][off:off + 48, :]
                    k_sb = qkv_sb[(1, pair)]
                    pO = ps_acc.tile([65, N], F32, tag="acc")
                    for j in range(8):
                        pS = ps_flow.tile([128, N], F32, tag="flow")
                        for nch in range(2):
                            nc.tensor.matmul(
                                pS[:, nch * 512:(nch + 1) * 512],
                                k_sb[off:off + 48, j * 128:(j + 1) * 128],
                                q_ap[:, nch * 512:(nch + 1) * 512],
                                start=True, stop=True)
                        pt = ptpool.tile([128, N], BF)
                        nc.scalar.activation(
                            pt[:], pS[:], _EXP, bias=0.0, scale=SCALE)
                        for nch in range(2):
                            nc.tensor.matmul(
                                pO[:, nch * 512:(nch + 1) * 512],
                                vt_all[j][:, h, 0:65],
                                pt[:, nch * 512:(nch + 1) * 512],
                                start=(j == 0), stop=(j == 7))
                    # rows 0..47 /= row 64: DVE reciprocal of the denom row,
                    # bf16 ones-matmul broadcast to 48 partitions, DVE mult.
                    # All row ops stay on partition 64 (no cross-lane moves).
                    inv = dpool.tile([65, N], F32, tag="rinv")
                    nc.vector.reciprocal(inv[64:65, :], pO[64:65, :])
                    invb = dpool.tile([65, N], FP16, tag="rinvb")
                    nc.vector.tensor_copy(invb[64:65, :], inv[64:65, :])
                    pb = ps_flow.tile([48, N], F32, tag="flow")
                    for nch in range(2):
                        nc.tensor.matmul(
                            pb[:, nch * 512:(nch + 1) * 512],
                            ones1[64:65, :],
                            invb[64:65, nch * 512:(nch + 1) * 512],
                            start=True, stop=True)
                    bc = dpool.tile([48, N], F32, tag="bc")
                    nc.vector.tensor_copy(bc[:], pb[:])
                    osb = opool.tile([48, N], FP16, tag=f"O{h}")
                    nc.vector.tensor_mul(osb[:], pO[0:48, :], bc[:])
                    O_sb[h] = osb

            # ---------------- out projection (bf16, K=48 per head)
            for m in range(NBLK):
                po = ps_acc.tile([128, N], F32, tag="acc")
                for h in range(HEADS):
                    lhsT = woT_t[h][:, m * 128:(m + 1) * 128]
                    for nch in range(2):
                        nc.tensor.matmul(
                            po[:, nch * 512:(nch + 1) * 512],
                            lhsT,
                            O_sb[h][:, nch * 512:(nch + 1) * 512],
                            start=(h == 0), stop=(h == HEADS - 1))
                ob = outpool.tile([128, N], F32)
                nc.vector.tensor_copy(ob[:], po[:])
                nc.sync.dma_start(out_d[m * 128:(m + 1) * 128, :], ob[:])

    if split_waits:
        _split_excess_waits(nc)
    return nc


_NC_CACHE = {}


def _get_nc():
    if "nc" not in _NC_CACHE:
        _NC_CACHE["nc"] = build_kernel()
    return _NC_CACHE["nc"]


# ---------------------------------------------------------------- host prep
def _conv_dw_np(x, dw):
    # x: (B, C, H, W) f32; dw: (C, 3, 3). padding=1 depthwise conv.
    Bx, Cx, Hx, Wx = x.shape
    xp = np.zeros((Bx, Cx, Hx + 2, Wx + 2), np.float32)
    xp[:, :, 1:Hx + 1, 1:Wx + 1] = x
    y = np.zeros((Bx, Cx, Hx, Wx), np.float32)
    for i in range(3):
        for j in range(3):
            y += dw[None, :, i, j, None, None] * \
                xp[:, :, i:i + Hx, j:j + Wx]
    return y


def _host_prep(inputs):
    x = np.ascontiguousarray(np.asarray(inputs["x"], np.float32))
    # diags[br, blk, c, tap, c] = dw_bf16[blk*128+c, tap]
    diags = np.zeros((3, NBLK, 128, 9, 128), BF16)
    A = np.zeros((3, C, 1), np.float32)
    D = np.zeros((3, C, 1), np.float32)
    pwT = np.zeros((2, C, NPAIR, 112), BF16)
    idx = np.arange(128)
    for br, p in enumerate(["q", "k", "v"]):
        dw = np.asarray(inputs[f"dw_{p}"], np.float32).reshape(C, 3, 3)
        dwb = dw.astype(BF16).astype(np.float32)
        y = _conv_dw_np(x, dwb)          # matches device conv (bf16 weights)
        m = y.astype(np.float64).mean(axis=(0, 2, 3))
        v = y.astype(np.float64).var(axis=(0, 2, 3))
        g = np.asarray(inputs[f"g_{p}"], np.float64)
        bb = np.asarray(inputs[f"b_{p}"], np.float64)
        a = g / np.sqrt(v + EPS)
        A[br, :, 0] = a.astype(np.float32)
        D[br, :, 0] = (bb - m * a).astype(np.float32)
        for blk in range(NBLK):
            for tap in range(9):
                diags[br, blk, idx, tap, idx] = \
                    dwb[blk * 128:(blk + 1) * 128, tap // 3, tap % 3]
        pwt = np.asarray(inputs[f"pw_{p}"], np.float32).T  # (c_in, c_out)
        if br < 2:
            for pair in range(NPAIR):
                pwT[br, :, pair, 0:48] = \
                    pwt[:, (2 * pair) * 48:(2 * pair + 1) * 48]
                pwT[br, :, pair, 64:112] = \
                    pwt[:, (2 * pair + 1) * 48:(2 * pair + 2) * 48]
        else:
            pwvT = np.ascontiguousarray(
                pwt.reshape(NBLK, 128, C)).astype(BF16)
    w_out = np.asarray(inputs["w_out"], np.float32)
    woT = np.ascontiguousarray(
        w_out.T.reshape(HEADS, HD, C)).astype(np.float16)
    return x, diags, A, D, pwT, pwvT, woT


def _make_in_maps(inputs):
    x, diags, A, D, pwT, pwvT, woT = _host_prep(inputs)
    in_maps = []
    for b in range(B):
        in_maps.append({
            "x": np.ascontiguousarray(x[b]),
            "diags": diags,
            "scaleA": A,
            "biasD": D,
            "pwT": pwT,
            "pwvT": pwvT,
            "woT": woT,
        })
    return in_maps


def kernel(**inputs) -> np.ndarray:
    in_maps = _make_in_maps(inputs)
    nc = _get_nc()
    res = run_bass_kernel_spmd(nc, in_maps, list(range(B)))
    out = np.stack([res.results[b]["out"] for b in range(B)])  # (B, C, N)

    o64 = out.astype(np.float64)
    m = o64.mean(axis=(0, 2))
    v = o64.var(axis=(0, 2))
    g = np.asarray(inputs["g_out"], np.float64)
    bb = np.asarray(inputs["b_out"], np.float64)
    res_f = (o64 - m[None, :, None]) / np.sqrt(v + EPS)[None, :, None] * \
        g[None, :, None] + bb[None, :, None]
    return res_f.reshape(B, C, H, W).astype(np.float32)


# revision 8
# speedup vs baseline: 1.8137x; 1.2556x over previous
"""Trainium2 Bass kernel for nn_ConvAttention (dwconv3x3->BN->GELU->1x1 conv
q/k/v branches, 8-head attention over 32x32 tokens, 1x1 out-proj, BN).

Sharding: data-parallel over batch B=8 across the 8 NeuronCores (one image
per core). The two training-mode BatchNorms couple cores across the batch:
  - the q/k/v-branch BN stats are computed on the HOST, exactly, from the
    inputs (the depthwise conv is recomputed cheaply in numpy just for the
    statistics; the device computes the conv for the actual data path), so
    the device kernel needs no cross-core communication at all;
  - the final BN is applied on the host after gathering (elementwise).

Device per-core pipeline (single NEFF launch), all matmuls bf16:
  x -> pad -> bf16 -> 9 accumulated diagonal matmuls per 128-channel block
  (depthwise conv on the PE) -> fused scale/bias+GELU on ACT (folded BN,
  bf16 out) -> bf16 pointwise matmuls. The v branch's pointwise runs in
  transposed form (lhsT = GELU output token chunk) so v^T tiles fall out of
  the matmul directly, with a zeros/ones tail appended per head for the
  softmax denominator row. Per head: S^T chunks = k_chunk^T q, P^T =
  exp(scale*S^T) on ACT -> bf16, O = [v^T|0|1]^T P^T accumulated over
  chunks (row 64 = denom), division via DVE reciprocal_approx_fast +
  bf16 ones-broadcast matmul + DVE multiply -> bf16 out-projection.
"""

import sys

sys.path.insert(0, "/opt/trn_rl_repo")

import numpy as np
import ml_dtypes

import concourse.bass as bass
import concourse.mybir as mybir
import concourse.tile as tile
from concourse.bass_utils import run_bass_kernel_spmd

BF16 = ml_dtypes.bfloat16
F32 = mybir.dt.float32
BF = mybir.dt.bfloat16
FP16 = mybir.dt.float16

B, C, H, W = 8, 384, 32, 32
N = H * W
HEADS, HD = 8, 48
SCALE = float(HD ** -0.5)
NBLK = C // 128          # 3 channel blocks
NPAIR = HEADS // 2       # 4 head pairs (M=112 pointwise blocks)
EPS = 1e-5
VSTR = 66                # per-head stride in the v^T tile (48 v + 16 z + 1 one + pad)

_GELU = mybir.ActivationFunctionType.Gelu
GELU_FUNC = [_GELU]  # sim_check overrides (CoreSim lacks Gelu)
_EXP = mybir.ActivationFunctionType.Exp


# ---------------------------------------------------------------- wait split
def _split_excess_waits(nc, max_waits=1):
    """Old walrus rejects >1 sync wait per instruction; hoist extras onto
    NoOps inserted just before, on the same engine (queue order preserved)."""
    n = 0
    for f in nc.m.functions:
        for bb in f.blocks:
            out, changed = [], False
            for inst in bb.instructions:
                si = inst.sync_info
                waits = list(si.on_wait) if si is not None else []
                if len(waits) > max_waits:
                    excess, keep = waits[:-max_waits], waits[-max_waits:]
                    for j, w in enumerate(excess):
                        nop = mybir.InstNoOp(
                            name=f"WSPLIT-{inst.name}-{j}", ins=[], outs=[])
                        nop.engine = inst.engine
                        nop.sync_info = mybir.SyncInfo(on_wait=[w], on_update=[])
                        out.append(nop)
                        n += 1
                    inst.sync_info = mybir.SyncInfo(
                        on_wait=keep, on_update=list(si.on_update))
                    changed = True
                out.append(inst)
            if changed:
                bb.instructions = out
    return n


# ---------------------------------------------------------------- builder
def build_kernel(split_waits=True):
    nc = bass.Bass("TRN2", target_bir_lowering=False, debug=False)

    x_d = nc.dram_tensor("x", [C, H, W], F32, kind="ExternalInput").ap()
    diag_d = nc.dram_tensor("diags", [3, NBLK, 128, 9, 128], BF,
                            kind="ExternalInput").ap()
    A_d = nc.dram_tensor("scaleA", [3, C, 1], F32, kind="ExternalInput").ap()
    D_d = nc.dram_tensor("biasD", [3, C, 1], F32, kind="ExternalInput").ap()
    pwT_d = nc.dram_tensor("pwT", [2, C, NPAIR, 112], BF,
                           kind="ExternalInput").ap()
    pwvT_d = nc.dram_tensor("pwvT", [NBLK, 128, C], BF,
                            kind="ExternalInput").ap()
    woT_d = nc.dram_tensor("woT", [HEADS, HD, C], FP16,
                           kind="ExternalInput").ap()
    out_d = nc.dram_tensor("out", [C, N], F32, kind="ExternalOutput").ap()

    with tile.TileContext(nc) as tc:
        from contextlib import ExitStack
        ctx = ExitStack()
        with ctx:
            cpool = ctx.enter_context(tc.tile_pool(name="consts", bufs=1))
            xpool = ctx.enter_context(tc.tile_pool(name="xin", bufs=2))
            padpool = ctx.enter_context(tc.tile_pool(name="pads", bufs=1))
            yhpool = ctx.enter_context(tc.tile_pool(name="yh", bufs=1))
            qkvpool = ctx.enter_context(tc.tile_pool(name="qkv", bufs=1))
            vtpool = ctx.enter_context(tc.tile_pool(name="vt", bufs=1))
            ptpool = ctx.enter_context(tc.tile_pool(name="pt", bufs=3))
            opool = ctx.enter_context(tc.tile_pool(name="osb", bufs=1))
            dpool = ctx.enter_context(tc.tile_pool(name="div", bufs=2))
            outpool = ctx.enter_context(tc.tile_pool(name="outsb", bufs=2))

            # PSUM: 8 banks of (128, 2KB); 2 rotating "flow" slots (S chunks,
            # div broadcasts) + 2 rotating "acc" slots (conv/pw/attention-O/
            # out-proj accumulators), 2 banks each.
            ps_acc = ctx.enter_context(
                tc.tile_pool(name="ps_acc", bufs=2, space="PSUM"))
            ps_flow = ctx.enter_context(
                tc.tile_pool(name="ps_flow", bufs=2, space="PSUM"))

            # ---------------- constants
            # ones row lives at partition 64 so it can pair with the softmax
            # denominator row (pO row 64) as matmul operands (32-aligned base).
            ones1 = cpool.tile([65, HD], FP16, tag="ones1")
            nc.gpsimd.memset(ones1[64:65, :], 1.0)

            # ---------------- inputs: x (pad to bf16), per-branch consts
            # branch emission order: v first so attention can start earliest;
            # interleave the first branch's conv weights with the x loads so
            # the PE can start as soon as (x0, diag_v0) land.
            BR_ORDER = (2, 0, 1)
            xpad, diag_t, A_t, D_t = {}, {}, {}, {}

            def load_diag(br, blk):
                t = cpool.tile([128, 9, 128], BF, tag=f"diag{br}_{blk}")
                nc.sync.dma_start(t[:], diag_d[br, blk])
                diag_t[(br, blk)] = t
                a = cpool.tile([128, 1], F32, tag=f"A{br}_{blk}")
                d = cpool.tile([128, 1], F32, tag=f"D{br}_{blk}")
                nc.sync.dma_start(
                    a[:], A_d[br, blk * 128:(blk + 1) * 128, :])
                nc.sync.dma_start(
                    d[:], D_d[br, blk * 128:(blk + 1) * 128, :])
                A_t[(br, blk)] = a
                D_t[(br, blk)] = d

            for blk in range(NBLK):
                xt = xpool.tile([128, H, W], F32)
                nc.sync.dma_start(xt[:], x_d[blk * 128:(blk + 1) * 128])
                load_diag(2, blk)
                xp = padpool.tile([128, H + 2, W + 2], BF, tag=f"xpad{blk}")
                nc.gpsimd.memset(xp[:], 0.0)
                nc.vector.tensor_copy(xp[:, 1:H + 1, 1:W + 1], xt[:])
                xpad[blk] = xp
            for br in (0, 1):
                for blk in range(NBLK):
                    load_diag(br, blk)
            pwvT_t = {}
            for kc in range(NBLK):
                t = cpool.tile([128, C], BF, tag=f"pwvT{kc}")
                nc.sync.dma_start(t[:], pwvT_d[kc])
                pwvT_t[kc] = t
            pwT_t = {}
            for br in range(2):
                for kc in range(NBLK):
                    t = cpool.tile([128, NPAIR, 112], BF,
                                   tag=f"pwT{br}_{kc}")
                    nc.sync.dma_start(
                        t[:], pwT_d[br, kc * 128:(kc + 1) * 128, :])
                    pwT_t[(br, kc)] = t
            woT_t = {}
            for h in range(HEADS):
                t = cpool.tile([HD, C], FP16, tag=f"woT{h}")
                nc.sync.dma_start(t[:], woT_d[h])
                woT_t[h] = t

            # ---------------- depthwise conv + BN + GELU (bf16 out)
            yh_t = {}

            def conv_branch(br):
                for blk in range(NBLK):
                    py = ps_acc.tile([128, N], F32, tag="acc")
                    dt = diag_t[(br, blk)]
                    for tap in range(9):
                        di, dj = tap // 3, tap % 3
                        for hf in range(2):
                            nc.tensor.matmul(
                                py[:, hf * 512:(hf + 1) * 512],
                                dt[:, tap, :],
                                xpad[blk][:, di + 16 * hf:di + 16 * hf + 16,
                                          dj:dj + W],
                                start=(tap == 0), stop=(tap == 8))
                    yh = yhpool.tile([128, N], BF, tag=f"yh{br}_{blk}")
                    nc.scalar.activation(
                        yh[:], py[:], GELU_FUNC[0],
                        bias=D_t[(br, blk)][:], scale=A_t[(br, blk)][:])
                    yh_t[(br, blk)] = yh

            # ---------------- v pointwise, transposed: vt[j] rows = tokens
            conv_branch(2)
            vt_all = {}
            for j in range(8):
                pv = ps_acc.tile([128, HEADS, HD], F32, tag="acc")
                for kc in range(NBLK):
                    nc.tensor.matmul(
                        pv[:, :, :],
                        yh_t[(2, kc)][:, j * 128:(j + 1) * 128],
                        pwvT_t[kc][:],
                        start=(kc == 0), stop=(kc == NBLK - 1))
                vt = vtpool.tile([128, HEADS, VSTR], BF, tag=f"vt{j}")
                nc.gpsimd.memset(vt[:, :, HD:64], 0.0)
                nc.gpsimd.memset(vt[:, :, 64:65], 1.0)
                nc.vector.tensor_copy(vt[:, :, 0:HD], pv[:])
                vt_all[j] = vt

            # ---------------- q,k pointwise (bf16, M=112 head-pairs)
            conv_branch(0)
            conv_branch(1)
            qkv_sb = {}

            def pw_pair(br, pair):
                pp = ps_acc.tile([112, N], F32, tag="acc")
                for kc in range(NBLK):
                    lhsT = pwT_t[(br, kc)][:, pair, :]
                    for nch in range(2):
                        nc.tensor.matmul(
                            pp[:, nch * 512:(nch + 1) * 512],
                            lhsT,
                            yh_t[(br, kc)][:, nch * 512:(nch + 1) * 512],
                            start=(kc == 0), stop=(kc == NBLK - 1))
                sb = qkvpool.tile([112, N], BF, tag=f"qkv{br}_{pair}")
                nc.vector.tensor_copy(sb[:], pp[:])
                qkv_sb[(br, pair)] = sb

            # ---------------- attention (interleave pw of later pairs)
            # Division pipeline: right after head h's O accumulation, copy pO
            # to SBUF (frees the PSUM acc slot so head h+2 can accumulate) and
            # start the slow DVE reciprocal. The division FINISHERS (broadcast
            # matmul + multiply) for head h are emitted during head h+1, so
            # the in-order PE queue never stalls waiting on a reciprocal.
            O_sb, Oc_t, invb_t = {}, {}, {}

            def div_finish(g):
                pb = ps_flow.tile([48, N], F32, tag="flow")
                for nch in range(2):
                    nc.tensor.matmul(
                        pb[:, nch * 512:(nch + 1) * 512],
                        ones1[64:65, :],
                        invb_t[g][64:65, nch * 512:(nch + 1) * 512],
                        start=True, stop=True)
                osb = opool.tile([48, N], FP16, tag=f"O{g}")
                nc.vector.tensor_mul(osb[:], Oc_t[g][0:48, :], pb[:])
                O_sb[g] = osb

            for pair in range(NPAIR):
                pw_pair(0, pair)
                pw_pair(1, pair)
                for hh in range(2):
                    h = 2 * pair + hh
                    off = 64 * hh
                    q_ap = qkv_sb[(0, pair)][off:off + 48, :]
                    k_sb = qkv_sb[(1, pair)]
                    pO = ps_acc.tile([65, N], F32, tag="acc")
                    for j in range(8):
                        pS = ps_flow.tile([128, N], F32, tag="flow")
                        for nch in range(2):
                            nc.tensor.matmul(
                                pS[:, nch * 512:(nch + 1) * 512],
                                k_sb[off:off + 48, j * 128:(j + 1) * 128],
                                q_ap[:, nch * 512:(nch + 1) * 512],
                                start=True, stop=True)
                        pt = ptpool.tile([128, N], BF)
                        nc.scalar.activation(
                            pt[:], pS[:], _EXP, bias=0.0, scale=SCALE)
                        for nch in range(2):
                            nc.tensor.matmul(
                                pO[:, nch * 512:(nch + 1) * 512],
                                vt_all[j][:, h, 0:65],
                                pt[:, nch * 512:(nch + 1) * 512],
                                start=(j == 0), stop=(j == 7))
                    # park O + denom in SBUF, finish the PREVIOUS head's
                    # division (its reciprocal is long done), then kick off
                    # this head's reciprocal. Ordering matters: the slow
                    # reciprocal must sit BEHIND the finisher in the in-order
                    # DVE queue, or it delays the flow-slot release.
                    oc = opool.tile([65, N], F32, tag=f"Oc{h}")
                    nc.vector.tensor_copy(oc[:], pO[:])
                    Oc_t[h] = oc
                    if h >= 1:
                        div_finish(h - 1)
                    inv = dpool.tile([65, N], F32, tag="rinv")
                    nc.vector.reciprocal(inv[64:65, :], oc[64:65, :])
                    invb = dpool.tile([65, N], FP16, tag=f"rinvb{h % 2}")
                    nc.vector.tensor_copy(invb[64:65, :], inv[64:65, :])
                    invb_t[h] = invb
            div_finish(HEADS - 1)

            # ---------------- out projection (bf16, K=48 per head)
            for m in range(NBLK):
                po = ps_acc.tile([128, N], F32, tag="acc")
                for h in range(HEADS):
                    lhsT = woT_t[h][:, m * 128:(m + 1) * 128]
                    for nch in range(2):
                        nc.tensor.matmul(
                            po[:, nch * 512:(nch + 1) * 512],
                            lhsT,
                            O_sb[h][:, nch * 512:(nch + 1) * 512],
                            start=(h == 0), stop=(h == HEADS - 1))
                ob = outpool.tile([128, N], F32)
                nc.vector.tensor_copy(ob[:], po[:])
                nc.sync.dma_start(out_d[m * 128:(m + 1) * 128, :], ob[:])

    if split_waits:
        _split_excess_waits(nc)
    return nc


_NC_CACHE = {}


def _get_nc():
    if "nc" not in _NC_CACHE:
        _NC_CACHE["nc"] = build_kernel()
    return _NC_CACHE["nc"]


# ---------------------------------------------------------------- host prep
def _conv_dw_np(x, dw):
    # x: (B, C, H, W) f32; dw: (C, 3, 3). padding=1 depthwise conv.
    Bx, Cx, Hx, Wx = x.shape
    xp = np.zeros((Bx, Cx, Hx + 2, Wx + 2), np.float32)
    xp[:, :, 1:Hx + 1, 1:Wx + 1] = x
    y = np.zeros((Bx, Cx, Hx, Wx), np.float32)
    for i in range(3):
        for j in range(3):
            y += dw[None, :, i, j, None, None] * \
                xp[:, :, i:i + Hx, j:j + Wx]
    return y


def _host_prep(inputs):
    x = np.ascontiguousarray(np.asarray(inputs["x"], np.float32))
    # diags[br, blk, c, tap, c] = dw_bf16[blk*128+c, tap]
    diags = np.zeros((3, NBLK, 128, 9, 128), BF16)
    A = np.zeros((3, C, 1), np.float32)
    D = np.zeros((3, C, 1), np.float32)
    pwT = np.zeros((2, C, NPAIR, 112), BF16)
    idx = np.arange(128)
    for br, p in enumerate(["q", "k", "v"]):
        dw = np.asarray(inputs[f"dw_{p}"], np.float32).reshape(C, 3, 3)
        dwb = dw.astype(BF16).astype(np.float32)
        y = _conv_dw_np(x, dwb)          # matches device conv (bf16 weights)
        m = y.astype(np.float64).mean(axis=(0, 2, 3))
        v = y.astype(np.float64).var(axis=(0, 2, 3))
        g = np.asarray(inputs[f"g_{p}"], np.float64)
        bb = np.asarray(inputs[f"b_{p}"], np.float64)
        a = g / np.sqrt(v + EPS)
        A[br, :, 0] = a.astype(np.float32)
        D[br, :, 0] = (bb - m * a).astype(np.float32)
        for blk in range(NBLK):
            for tap in range(9):
                diags[br, blk, idx, tap, idx] = \
                    dwb[blk * 128:(blk + 1) * 128, tap // 3, tap % 3]
        pwt = np.asarray(inputs[f"pw_{p}"], np.float32).T  # (c_in, c_out)
        if br < 2:
            for pair in range(NPAIR):
                pwT[br, :, pair, 0:48] = \
                    pwt[:, (2 * pair) * 48:(2 * pair + 1) * 48]
                pwT[br, :, pair, 64:112] = \
                    pwt[:, (2 * pair + 1) * 48:(2 * pair + 2) * 48]
        else:
            pwvT = np.ascontiguousarray(
                pwt.reshape(NBLK, 128, C)).astype(BF16)
    w_out = np.asarray(inputs["w_out"], np.float32)
    woT = np.ascontiguousarray(
        w_out.T.reshape(HEADS, HD, C)).astype(np.float16)
    return x, diags, A, D, pwT, pwvT, woT


def _make_in_maps(inputs):
    x, diags, A, D, pwT, pwvT, woT = _host_prep(inputs)
    in_maps = []
    for b in range(B):
        in_maps.append({
            "x": np.ascontiguousarray(x[b]),
            "diags": diags,
            "scaleA": A,
            "biasD": D,
            "pwT": pwT,
            "pwvT": pwvT,
            "woT": woT,
        })
    return in_maps


def kernel(**inputs) -> np.ndarray:
    in_maps = _make_in_maps(inputs)
    nc = _get_nc()
    res = run_bass_kernel_spmd(nc, in_maps, list(range(B)))
    out = np.stack([res.results[b]["out"] for b in range(B)])  # (B, C, N)

    o64 = out.astype(np.float64)
    m = o64.mean(axis=(0, 2))
    v = o64.var(axis=(0, 2))
    g = np.asarray(inputs["g_out"], np.float64)
    bb = np.asarray(inputs["b_out"], np.float64)
    res_f = (o64 - m[None, :, None]) / np.sqrt(v + EPS)[None, :, None] * \
        g[None, :, None] + bb[None, :, None]
    return res_f.reshape(B, C, H, W).astype(np.float32)


# revision 14
# speedup vs baseline: 1.8831x; 1.0383x over previous
"""Trainium2 Bass kernel for nn_ConvAttention (dwconv3x3->BN->GELU->1x1 conv
q/k/v branches, 8-head attention over 32x32 tokens, 1x1 out-proj, BN).

Sharding: data-parallel over batch B=8 across the 8 NeuronCores (one image
per core). The two training-mode BatchNorms couple cores across the batch:
  - the q/k/v-branch BN stats are computed on the HOST, exactly, from the
    inputs (the depthwise conv is recomputed cheaply in numpy just for the
    statistics; the device computes the conv for the actual data path), so
    the device kernel needs no cross-core communication at all;
  - the final BN is applied on the host after gathering (elementwise).

Device per-core pipeline (single NEFF launch), all matmuls bf16:
  x -> pad -> bf16 -> 9 accumulated diagonal matmuls per 128-channel block
  (depthwise conv on the PE) -> fused scale/bias+GELU on ACT (folded BN,
  bf16 out) -> bf16 pointwise matmuls. The v branch's pointwise runs in
  transposed form (lhsT = GELU output token chunk) so v^T tiles fall out of
  the matmul directly, with a zeros/ones tail appended per head for the
  softmax denominator row. Per head: S^T chunks = k_chunk^T q, P^T =
  exp(scale*S^T) on ACT -> bf16, O = [v^T|0|1]^T P^T accumulated over
  chunks (row 64 = denom), division via DVE reciprocal_approx_fast +
  bf16 ones-broadcast matmul + DVE multiply -> bf16 out-projection.
"""

import sys

sys.path.insert(0, "/opt/trn_rl_repo")

import numpy as np
import ml_dtypes

import concourse.bass as bass
import concourse.mybir as mybir
import concourse.tile as tile
from concourse.bass_utils import run_bass_kernel_spmd

BF16 = ml_dtypes.bfloat16
F32 = mybir.dt.float32
BF = mybir.dt.bfloat16
FP16 = mybir.dt.float16

B, C, H, W = 8, 384, 32, 32
N = H * W
HEADS, HD = 8, 48
SCALE = float(HD ** -0.5)
NBLK = C // 128          # 3 channel blocks
NPAIR = HEADS // 2       # 4 head pairs (M=112 pointwise blocks)
EPS = 1e-5
VSTR = 66                # per-head stride in the v^T tile (48 v + 16 z + 1 one + pad)

_GELU = mybir.ActivationFunctionType.Gelu
GELU_FUNC = [_GELU]  # sim_check overrides (CoreSim lacks Gelu)
_EXP = mybir.ActivationFunctionType.Exp


# ---------------------------------------------------------------- wait split
def _split_excess_waits(nc, max_waits=1):
    """Old walrus rejects >1 sync wait per instruction; hoist extras onto
    NoOps inserted just before, on the same engine (queue order preserved)."""
    n = 0
    for f in nc.m.functions:
        for bb in f.blocks:
            out, changed = [], False
            for inst in bb.instructions:
                si = inst.sync_info
                waits = list(si.on_wait) if si is not None else []
                if len(waits) > max_waits:
                    excess, keep = waits[:-max_waits], waits[-max_waits:]
                    for j, w in enumerate(excess):
                        nop = mybir.InstNoOp(
                            name=f"WSPLIT-{inst.name}-{j}", ins=[], outs=[])
                        nop.engine = inst.engine
                        nop.sync_info = mybir.SyncInfo(on_wait=[w], on_update=[])
                        out.append(nop)
                        n += 1
                    inst.sync_info = mybir.SyncInfo(
                        on_wait=keep, on_update=list(si.on_update))
                    changed = True
                out.append(inst)
            if changed:
                bb.instructions = out
    return n


# ---------------------------------------------------------------- builder
def build_kernel(split_waits=True):
    nc = bass.Bass("TRN2", target_bir_lowering=False, debug=False)

    x_d = nc.dram_tensor("x", [C, H, W], F32, kind="ExternalInput").ap()
    diag_d = nc.dram_tensor("diags", [3, NBLK, 128, 9, 128], BF,
                            kind="ExternalInput").ap()
    A_d = nc.dram_tensor("scaleA", [3, C, 1], F32, kind="ExternalInput").ap()
    D_d = nc.dram_tensor("biasD", [3, C, 1], F32, kind="ExternalInput").ap()
    pwT_d = nc.dram_tensor("pwT", [2, C, NPAIR, 112], BF,
                           kind="ExternalInput").ap()
    pwvT_d = nc.dram_tensor("pwvT", [NBLK, 128, C], BF,
                            kind="ExternalInput").ap()
    woT_d = nc.dram_tensor("woT", [HEADS, HD, C], FP16,
                           kind="ExternalInput").ap()
    out_d = nc.dram_tensor("out", [C, N], F32, kind="ExternalOutput").ap()
    binv_d = nc.dram_tensor("binv", [HEADS, 1, N], FP16, kind="Internal").ap()

    with tile.TileContext(nc) as tc:
        from contextlib import ExitStack
        ctx = ExitStack()
        with ctx:
            cpool = ctx.enter_context(tc.tile_pool(name="consts", bufs=1))
            xpool = ctx.enter_context(tc.tile_pool(name="xin", bufs=2))
            padpool = ctx.enter_context(tc.tile_pool(name="pads", bufs=1))
            yhpool = ctx.enter_context(tc.tile_pool(name="yh", bufs=1))
            qkvpool = ctx.enter_context(tc.tile_pool(name="qkv", bufs=1))
            vtpool = ctx.enter_context(tc.tile_pool(name="vt", bufs=1))
            ptpool = ctx.enter_context(tc.tile_pool(name="pt", bufs=3))
            opool = ctx.enter_context(tc.tile_pool(name="osb", bufs=1))
            dpool = ctx.enter_context(tc.tile_pool(name="div", bufs=2))
            outpool = ctx.enter_context(tc.tile_pool(name="outsb", bufs=2))

            # PSUM: 8 banks of (128, 2KB); 2 rotating "flow" slots (S chunks,
            # div broadcasts) + 2 rotating "acc" slots (conv/pw/attention-O/
            # out-proj accumulators), 2 banks each.
            ps_acc = ctx.enter_context(
                tc.tile_pool(name="ps_acc", bufs=2, space="PSUM"))
            ps_flow = ctx.enter_context(
                tc.tile_pool(name="ps_flow", bufs=2, space="PSUM"))

            # ---------------- inputs: x (pad to bf16), per-branch consts
            # branch emission order: v first so attention can start earliest;
            # interleave the first branch's conv weights with the x loads so
            # the PE can start as soon as (x0, diag_v0) land.
            BR_ORDER = (2, 0, 1)
            xpad, diag_t, A_t, D_t = {}, {}, {}, {}

            def load_diag(br, blk):
                t = cpool.tile([128, 9, 128], BF, tag=f"diag{br}_{blk}")
                nc.sync.dma_start(t[:], diag_d[br, blk])
                diag_t[(br, blk)] = t
                a = cpool.tile([128, 1], F32, tag=f"A{br}_{blk}")
                d = cpool.tile([128, 1], F32, tag=f"D{br}_{blk}")
                nc.sync.dma_start(
                    a[:], A_d[br, blk * 128:(blk + 1) * 128, :])
                nc.sync.dma_start(
                    d[:], D_d[br, blk * 128:(blk + 1) * 128, :])
                A_t[(br, blk)] = a
                D_t[(br, blk)] = d

            for blk in range(NBLK):
                xt = xpool.tile([128, H, W], F32)
                nc.sync.dma_start(xt[:], x_d[blk * 128:(blk + 1) * 128])
                load_diag(2, blk)
                xp = padpool.tile([128, H + 2, W + 2], BF, tag=f"xpad{blk}")
                nc.gpsimd.memset(xp[:], 0.0)
                nc.vector.tensor_copy(xp[:, 1:H + 1, 1:W + 1], xt[:])
                xpad[blk] = xp
            # pw weights next in queue order: pw_v is needed right after the
            # v conv; the q/k conv weights can land later.
            pwvT_t = {}
            for kc in range(NBLK):
                t = cpool.tile([128, C], BF, tag=f"pwvT{kc}")
                nc.sync.dma_start(t[:], pwvT_d[kc])
                pwvT_t[kc] = t
            for br in (0, 1):
                for blk in range(NBLK):
                    load_diag(br, blk)
            pwT_t = {}
            for br in range(2):
                for kc in range(NBLK):
                    t = cpool.tile([128, NPAIR, 112], BF,
                                   tag=f"pwT{br}_{kc}")
                    nc.sync.dma_start(
                        t[:], pwT_d[br, kc * 128:(kc + 1) * 128, :])
                    pwT_t[(br, kc)] = t
            woT_t = {}
            for h in range(HEADS):
                t = cpool.tile([HD, C], FP16, tag=f"woT{h}")
                nc.sync.dma_start(t[:], woT_d[h])
                woT_t[h] = t

            # ---------------- depthwise conv + BN + GELU (bf16 out)
            yh_t = {}

            def conv_branch(br):
                for blk in range(NBLK):
                    py = ps_acc.tile([128, N], F32, tag="acc")
                    dt = diag_t[(br, blk)]
                    for tap in range(9):
                        di, dj = tap // 3, tap % 3
                        for hf in range(2):
                            nc.tensor.matmul(
                                py[:, hf * 512:(hf + 1) * 512],
                                dt[:, tap, :],
                                xpad[blk][:, di + 16 * hf:di + 16 * hf + 16,
                                          dj:dj + W],
                                start=(tap == 0), stop=(tap == 8))
                    yh = yhpool.tile([128, N], BF, tag=f"yh{br}_{blk}")
                    nc.scalar.activation(
                        yh[:], py[:], GELU_FUNC[0],
                        bias=D_t[(br, blk)][:], scale=A_t[(br, blk)][:])
                    yh_t[(br, blk)] = yh

            # ---------------- v pointwise, transposed: vt[j] rows = tokens
            conv_branch(2)
            vt_all = {}
            for j in range(8):
                pv = ps_acc.tile([128, HEADS, HD], F32, tag="acc")
                for kc in range(NBLK):
                    nc.tensor.matmul(
                        pv[:, :, :],
                        yh_t[(2, kc)][:, j * 128:(j + 1) * 128],
                        pwvT_t[kc][:],
                        start=(kc == 0), stop=(kc == NBLK - 1))
                vt = vtpool.tile([128, HEADS, VSTR], BF, tag=f"vt{j}")
                nc.gpsimd.memset(vt[:, :, HD:64], 0.0)
                nc.gpsimd.memset(vt[:, :, 64:65], 1.0)
                nc.vector.tensor_copy(vt[:, :, 0:HD], pv[:])
                vt_all[j] = vt

            # ---------------- q,k pointwise (bf16, M=112 head-pairs)
            conv_branch(0)
            conv_branch(1)
            qkv_sb = {}

            def pw_pair(br, pair):
                pp = ps_acc.tile([112, N], F32, tag="acc")
                for kc in range(NBLK):
                    lhsT = pwT_t[(br, kc)][:, pair, :]
                    for nch in range(2):
                        nc.tensor.matmul(
                            pp[:, nch * 512:(nch + 1) * 512],
                            lhsT,
                            yh_t[(br, kc)][:, nch * 512:(nch + 1) * 512],
                            start=(kc == 0), stop=(kc == NBLK - 1))
                sb = qkvpool.tile([112, N], BF, tag=f"qkv{br}_{pair}")
                nc.vector.tensor_copy(sb[:], pp[:])
                qkv_sb[(br, pair)] = sb

            # ---------------- attention (interleave pw of later pairs)
            # Division pipeline: right after head h's O accumulation, copy pO
            # to SBUF (frees the PSUM acc slot so head h+2 can accumulate) and
            # start the slow DVE reciprocal. The division FINISHERS (broadcast
            # matmul + multiply) for head h are emitted during head h+1, so
            # the in-order PE queue never stalls waiting on a reciprocal.
            O_sb, Oc_t, bcS_t = {}, {}, {}

            def div_finish(g):
                osb = opool.tile([48, N], FP16, tag=f"O{g}")
                nc.vector.tensor_mul(osb[:], Oc_t[g][0:48, :], bcS_t[g][:])
                O_sb[g] = osb

            for pair in range(NPAIR):
                pw_pair(0, pair)
                pw_pair(1, pair)
                for hh in range(2):
                    h = 2 * pair + hh
                    off = 64 * hh
                    q_ap = qkv_sb[(0, pair)][off:off + 48, :]
                    k_sb = qkv_sb[(1, pair)]
                    pO = ps_acc.tile([65, N], F32, tag="acc")
                    for j in range(8):
                        pS = ps_flow.tile([128, N], F32, tag="flow")
                        for nch in range(2):
                            nc.tensor.matmul(
                                pS[:, nch * 512:(nch + 1) * 512],
                                k_sb[off:off + 48, j * 128:(j + 1) * 128],
                                q_ap[:, nch * 512:(nch + 1) * 512],
                                start=True, stop=True)
                        pt = ptpool.tile([128, N], BF)
                        nc.scalar.activation(
                            pt[:], pS[:], _EXP, bias=0.0, scale=SCALE)
                        for nch in range(2):
                            nc.tensor.matmul(
                                pO[:, nch * 512:(nch + 1) * 512],
                                vt_all[j][:, h, 0:65],
                                pt[:, nch * 512:(nch + 1) * 512],
                                start=(j == 0), stop=(j == 7))
                    # park O + denom in SBUF, finish the PREVIOUS head's
                    # division (its reciprocal is long done), then kick off
                    # this head's reciprocal. Ordering matters: the slow
                    # reciprocal must sit BEHIND the finisher in the in-order
                    # DVE queue, or it delays the flow-slot release.
                    oc = opool.tile([65, N], F32, tag=f"Oc{h}")
                    nc.vector.tensor_copy(oc[:], pO[:])
                    Oc_t[h] = oc
                    if h >= 1:
                        div_finish(h - 1)
                    inv = dpool.tile([65, N], F32, tag="rinv")
                    nc.vector.reciprocal(inv[64:65, :], oc[64:65, :])
                    invb = dpool.tile([65, N], FP16, tag=f"rinvb{h % 2}")
                    nc.vector.tensor_copy(invb[64:65, :], inv[64:65, :])
                    # replicate 1/denom to 48 partitions via a DRAM bounce:
                    # SBUF partition-stride-0 is rejected, but a DRAM source
                    # row can be read 48x (stride-0 on a DRAM dim). Both DMAs
                    # share the in-order queue, so write-then-read is safe.
                    # Cheaper than a PE ones-matmul and needs no PSUM slot.
                    nc.sync.dma_start(binv_d[h], invb[64:65, :])
                    bcS = dpool.tile([48, N], FP16, tag=f"bcS{h % 2}")
                    src, dst = bass.broadcast_tensor_aps(
                        binv_d[h], bcS[:])
                    nc.sync.dma_start(dst, src)
                    bcS_t[h] = bcS
            div_finish(HEADS - 1)

            # ---------------- out projection (bf16, K=48 per head)
            for m in range(NBLK):
                po = ps_acc.tile([128, N], F32, tag="acc")
                for h in range(HEADS):
                    lhsT = woT_t[h][:, m * 128:(m + 1) * 128]
                    for nch in range(2):
                        nc.tensor.matmul(
                            po[:, nch * 512:(nch + 1) * 512],
                            lhsT,
                            O_sb[h][:, nch * 512:(nch + 1) * 512],
                            start=(h == 0), stop=(h == HEADS - 1))
                ob = outpool.tile([128, N], F32)
                nc.vector.tensor_copy(ob[:], po[:])
                nc.sync.dma_start(out_d[m * 128:(m + 1) * 128, :], ob[:])

    if split_waits:
        _split_excess_waits(nc)
    return nc


_NC_CACHE = {}


def _get_nc():
    if "nc" not in _NC_CACHE:
        _NC_CACHE["nc"] = build_kernel()
    return _NC_CACHE["nc"]


# ---------------------------------------------------------------- host prep
def _conv_dw_np(x, dw):
    # x: (B, C, H, W) f32; dw: (C, 3, 3). padding=1 depthwise conv.
    Bx, Cx, Hx, Wx = x.shape
    xp = np.zeros((Bx, Cx, Hx + 2, Wx + 2), np.float32)
    xp[:, :, 1:Hx + 1, 1:Wx + 1] = x
    y = np.zeros((Bx, Cx, Hx, Wx), np.float32)
    for i in range(3):
        for j in range(3):
            y += dw[None, :, i, j, None, None] * \
                xp[:, :, i:i + Hx, j:j + Wx]
    return y


def _host_prep(inputs):
    x = np.ascontiguousarray(np.asarray(inputs["x"], np.float32))
    # diags[br, blk, c, tap, c] = dw_bf16[blk*128+c, tap]
    diags = np.zeros((3, NBLK, 128, 9, 128), BF16)
    A = np.zeros((3, C, 1), np.float32)
    D = np.zeros((3, C, 1), np.float32)
    pwT = np.zeros((2, C, NPAIR, 112), BF16)
    idx = np.arange(128)
    for br, p in enumerate(["q", "k", "v"]):
        dw = np.asarray(inputs[f"dw_{p}"], np.float32).reshape(C, 3, 3)
        dwb = dw.astype(BF16).astype(np.float32)
        y = _conv_dw_np(x, dwb)          # matches device conv (bf16 weights)
        m = y.astype(np.float64).mean(axis=(0, 2, 3))
        v = y.astype(np.float64).var(axis=(0, 2, 3))
        g = np.asarray(inputs[f"g_{p}"], np.float64)
        bb = np.asarray(inputs[f"b_{p}"], np.float64)
        a = g / np.sqrt(v + EPS)
        A[br, :, 0] = a.astype(np.float32)
        D[br, :, 0] = (bb - m * a).astype(np.float32)
        for blk in range(NBLK):
            for tap in range(9):
                diags[br, blk, idx, tap, idx] = \
                    dwb[blk * 128:(blk + 1) * 128, tap // 3, tap % 3]
        pwt = np.asarray(inputs[f"pw_{p}"], np.float32).T  # (c_in, c_out)
        if br < 2:
            for pair in range(NPAIR):
                pwT[br, :, pair, 0:48] = \
                    pwt[:, (2 * pair) * 48:(2 * pair + 1) * 48]
                pwT[br, :, pair, 64:112] = \
                    pwt[:, (2 * pair + 1) * 48:(2 * pair + 2) * 48]
        else:
            pwvT = np.ascontiguousarray(
                pwt.reshape(NBLK, 128, C)).astype(BF16)
    w_out = np.asarray(inputs["w_out"], np.float32)
    woT = np.ascontiguousarray(
        w_out.T.reshape(HEADS, HD, C)).astype(np.float16)
    return x, diags, A, D, pwT, pwvT, woT


def _make_in_maps(inputs):
    x, diags, A, D, pwT, pwvT, woT = _host_prep(inputs)
    in_maps = []
    for b in range(B):
        in_maps.append({
            "x": np.ascontiguousarray(x[b]),
            "diags": diags,
            "scaleA": A,
            "biasD": D,
            "pwT": pwT,
            "pwvT": pwvT,
            "woT": woT,
        })
    return in_maps


def kernel(**inputs) -> np.ndarray:
    in_maps = _make_in_maps(inputs)
    nc = _get_nc()
    res = run_bass_kernel_spmd(nc, in_maps, list(range(B)))
    out = np.stack([res.results[b]["out"] for b in range(B)])  # (B, C, N)

    o64 = out.astype(np.float64)
    m = o64.mean(axis=(0, 2))
    v = o64.var(axis=(0, 2))
    g = np.asarray(inputs["g_out"], np.float64)
    bb = np.asarray(inputs["b_out"], np.float64)
    res_f = (o64 - m[None, :, None]) / np.sqrt(v + EPS)[None, :, None] * \
        g[None, :, None] + bb[None, :, None]
    return res_f.reshape(B, C, H, W).astype(np.float32)
